# revision 1
# baseline (speedup 1.0000x reference)
"""Trainium2 Bass kernel: 12-head attention block (qkv proj -> softmax attn -> fc).

Reference semantics (B=32, S=577, D=768, H=12, Dh=64):
    qkv = x @ w_qkv + b_qkv
    q, k, v = split(qkv); attn = softmax(q k^T / 8) v
    out = attn @ w_fc + b_fc

Sharding: data-parallel over batch across 8 NeuronCores (4 images per core),
weights replicated, no collectives. Compute in bf16 with fp32 PSUM accumulation.

Layout strategy per core (all matmuls contract over the partition dim):
  - xT [768, 577] built from x via PE transposes (fp32 transpose mode).
  - qkT [1536, 577] = w_qkv[:, :1536]^T . xT  (w_qkv stationary in natural layout).
  - v   [577, 768]  = xT^T . w_qkv[:, 1536:]  (natural layout, per-head ones column
    appended so attention row-sums fall out of the attn@v matmul for free).
  - scoresT[sk, sq] = kT_h^T . qT_h; heads paired even/odd so their K=64
    matmuls land on disjoint PE row groups and run concurrently. exp on
    ScalarE (scale=1/8 folded into exp; no max subtraction -- scores are O(1)
    here, exp is safe in fp32).
  - attn_outT[65, sq] = (v_h|1)^T . expT ; row 64 = softmax denominators.
  - normalize: staged reciprocal_approx_fast + gpsimd partition_broadcast +
    in-place DVE multiply on the transposed attention output attn_T [768,577].
  - fc: out[s, :] = attn_T_k^T . w_fc_k (natural w_fc), + b_fc broadcast.

Scheduling (the part that matters for speed): attention alternates
PE-light/ScalarE-heavy (scores+exp) with PE-heavy (attn@v) phases, so all
independent PE work -- this batch's late qkT tiles, the previous batch's fc,
the next batch's qkT/v/xT -- is woven as "filler" between scores sk-groups.
That keeps the TensorE stream dense, which both hides the exp latency and
keeps the PE HAM clock-gate at full rate (idle gaps re-throttle it to half
clock). Weights are loaded via gpsimd casting DMA (f32->bf16 in flight, no
staging) with w_fc queued last; startup x DMAs split across the sync and
scalar HWDGE queues.
PSUM: two 2-slot pools (scores vs everything else), 8 banks total.
"""

import os
import sys

import numpy as np

for _p in ("/opt/trn_rl_repo", "/root/.axon_site/_ro/trn_rl_repo"):
    if os.path.isdir(_p) and _p not in sys.path:
        sys.path.insert(0, _p)

import concourse.bass as bass  # noqa: E402
import concourse.tile as tile  # noqa: E402
from concourse import bacc, mybir  # noqa: E402
from concourse.bass_utils import run_bass_kernel_spmd  # noqa: E402
from concourse.masks import make_identity  # noqa: E402

F32 = mybir.dt.float32
BF16 = mybir.dt.bfloat16

B, S, D = 32, 577, 768
H, DH = 12, 64
NCORES = 8
NB = B // NCORES  # 4 batch images per core
SCALE = DH**-0.5  # 0.125
NKT = D // 128  # 6 contraction tiles of 128
S_TILES = [(0, 128), (128, 128), (256, 128), (384, 128), (512, 65)]
CH_S = [(0, 512), (512, 65)]  # 577 split at PSUM-bank boundary
CH_D = [(0, 512), (512, 256)]  # 768 split at PSUM-bank boundary
EXP = mybir.ActivationFunctionType.Exp
IDENT = mybir.ActivationFunctionType.Identity


def build_nc():
    nc = bacc.Bacc(None)
    x_ext = nc.declare_dram_parameter("x", [NB, S, D], F32, isOutput=False)
    wqkv_ext = nc.declare_dram_parameter("w_qkv", [D, 3 * D], F32, isOutput=False)
    bqkv_ext = nc.declare_dram_parameter("b_qkv", [3 * D], F32, isOutput=False)
    wfc_ext = nc.declare_dram_parameter("w_fc", [D, D], F32, isOutput=False)
    bfc_ext = nc.declare_dram_parameter("b_fc", [D], F32, isOutput=False)
    out_ext = nc.declare_dram_parameter("out", [NB, S, D], F32, isOutput=True)

    with tile.TileContext(nc) as tc:
        with (
            tc.tile_pool(name="const", bufs=1) as cpool,
            tc.tile_pool(name="x", bufs=2) as x_pool,
            tc.tile_pool(name="xT", bufs=2) as xT_pool,
            tc.tile_pool(name="qkT", bufs=2) as qkT_pool,
            tc.tile_pool(name="v", bufs=2) as v_pool,
            tc.tile_pool(name="expT", bufs=5) as expT_pool,
            tc.tile_pool(name="attnT", bufs=2) as attnT_pool,
            tc.tile_pool(name="small", bufs=3) as small_pool,
            tc.tile_pool(name="osb", bufs=3) as osb_pool,
            tc.tile_pool(name="psS", bufs=2, space="PSUM") as psS,
            tc.tile_pool(name="psW", bufs=2, space="PSUM") as psW,
        ):
            # ---- batch-0 x DMA first so PE transposes can start while the
            # (larger) weight DMAs + casts stream in behind it ----

            # ---- constants / weights (once) ----
            identity = cpool.tile([128, 128], F32)
            make_identity(nc, identity[:])
            ones = cpool.tile([1, 128], F32)
            nc.vector.memset(ones[:], 1.0)

            b_qk = cpool.tile([128, H], F32)  # per-partition bias for qkT tiles
            brow_v = cpool.tile([1, D], F32)
            nc.sync.dma_start(brow_v[:], bqkv_ext[None, 2 * D : 3 * D])
            brow_fc = cpool.tile([1, D], F32)
            nc.sync.dma_start(brow_fc[:], bfc_ext[None, :])

            # broadcast biases to all 128 partitions via K=1 matmul
            b_v_bc = cpool.tile([128, D], F32)
            b_fc_bc = cpool.tile([128, D], F32)
            for row, bc in ((brow_v, b_v_bc), (brow_fc, b_fc_bc)):
                pb = psW.tile([128, D], F32, tag="psW", name="pb")
                for c0, cl in CH_D:
                    nc.tensor.matmul(
                        pb[:, c0 : c0 + cl],
                        lhsT=ones[0:1, 0:128],
                        rhs=row[0:1, c0 : c0 + cl],
                        start=True,
                        stop=True,
                    )
                nc.vector.tensor_copy(bc[:], pb[:])

            # ---- per-batch emission helpers ----
            x_t, xT_t, qkT_t = {}, {}, {}

            def emit_x_dma(b, split=False):
                x_t[b] = x_pool.tile([128, 5 * D], F32, tag="x", name="x_all")
                for si, (s0, psl) in enumerate(S_TILES):
                    eng = nc.scalar if (split and si % 2) else nc.sync
                    eng.dma_start(
                        x_t[b][0:psl, si * D : (si + 1) * D],
                        x_ext[b, s0 : s0 + psl, :],
                    )

            def emit_xT(b):
                xT_t[b] = [
                    xT_pool.tile([128, S], BF16, tag=f"xT{dk}", name=f"xT{dk}")
                    for dk in range(NKT)
                ]
                for dk in range(NKT):
                    px = psW.tile([128, S], F32, tag="psW", name="px")
                    for si, (s0, psl) in enumerate(S_TILES):
                        nc.tensor.transpose(
                            px[:, s0 : s0 + psl],
                            x_t[b][0:psl, si * D + dk * 128 : si * D + (dk + 1) * 128],
                            identity[0:psl, 0:psl],
                        )
                    nc.vector.tensor_copy(xT_t[b][dk][:], px[:])

            def emit_qkT_mtile(b, m):
                # qkT tile m holds rows [m*128, (m+1)*128) = q or k of 2 heads;
                # per-m tiles so woven writes don't false-depend on reads
                if b not in qkT_t:
                    qkT_t[b] = {}
                qkT_t[b][m] = qkT_pool.tile(
                    [128, S], BF16, tag=f"qkT{m}", name=f"qkT{m}"
                )
                pqk = psW.tile([128, S], F32, tag="psW", name="pqk")
                for k in range(NKT):
                    for c0, cl in CH_S:
                        nc.tensor.matmul(
                            pqk[:, c0 : c0 + cl],
                            lhsT=w_qkv_k[k][:, m * 128 : (m + 1) * 128],
                            rhs=xT_t[b][k][:, c0 : c0 + cl],
                            start=(k == 0),
                            stop=(k == NKT - 1),
                        )
                nc.scalar.activation(
                    qkT_t[b][m][:], pqk[:], IDENT, bias=b_qk[:, m : m + 1]
                )

            def emit_v(b):
                # v natural [577, 768] + per-head ones column (65 floats per head)
                v_all = v_pool.tile(
                    [128, 5 * H * (DH + 1)], BF16, tag="v", name="v_all"
                )
                v4 = v_all[:].rearrange("p (s h e) -> p s h e", s=5, h=H)
                nc.vector.memset(v4[:, :, :, DH : DH + 1], 1.0)
                for si, (s0, psl) in enumerate(S_TILES):
                    pv = psW.tile([128, D], F32, tag="psW", name="pv")
                    for k in range(NKT):
                        for c0, cl in CH_D:
                            nc.tensor.matmul(
                                pv[0:psl, c0 : c0 + cl],
                                lhsT=xT_t[b][k][:, s0 : s0 + psl],
                                rhs=w_qkv_k[k][:, 2 * D + c0 : 2 * D + c0 + cl],
                                start=(k == 0),
                                stop=(k == NKT - 1),
                            )
                    nc.vector.tensor_add(
                        v4[0:psl, si, :, 0:DH],
                        pv[0:psl, :].rearrange("p (h e) -> p h e", h=H),
                        b_v_bc[0:psl, :].rearrange("p (h e) -> p h e", h=H),
                    )
                return v_all

            def emit_scores(p, qkT_all, expT, try_fill=None):
                heads = (2 * p, 2 * p + 1)
                for h in heads:
                    expT[h] = expT_pool.tile(
                        [128, 5 * S], BF16, tag="expT", name=f"expT{h % 2}"
                    )
                for si, (s0, psl) in enumerate(S_TILES):
                    psc = {}
                    for h in heads:
                        psc[h] = psS.tile([128, S], F32, tag="psS", name=f"psc{h % 2}")
                    for c0, cl in CH_S:
                        for h in heads:
                            hoff = (h % 2) * 64
                            qm, km = h // 2, NKT + h // 2
                            nc.tensor.matmul(
                                psc[h][0:psl, c0 : c0 + cl],
                                lhsT=qkT_all[km][hoff : hoff + 64, s0 : s0 + psl],
                                rhs=qkT_all[qm][hoff : hoff + 64, c0 : c0 + cl],
                                start=True,
                                stop=True,
                            )
                    for h in heads:
                        nc.scalar.activation(
                            expT[h][0:psl, si * S : (si + 1) * S],
                            psc[h][0:psl, :],
                            EXP,
                            scale=float(SCALE),
                        )
                    if try_fill is not None:
                        try_fill()

            def emit_attnv(p, v_all, attnT_all, expT):
                heads = (2 * p, 2 * p + 1)
                rinv = {}
                for h in heads:
                    hoff = (h % 2) * 64
                    # attn_outT [65, 577]: rows 0:64 = out^T unnorm, row 64 = sums
                    po = psW.tile([65, S], F32, tag="psW", name="po")
                    for si, (s0, psl) in enumerate(S_TILES):
                        for c0, cl in CH_S:
                            nc.tensor.matmul(
                                po[:, c0 : c0 + cl],
                                lhsT=v_all[
                                    0:psl,
                                    si * H * (DH + 1)
                                    + h * (DH + 1) : si * H * (DH + 1)
                                    + (h + 1) * (DH + 1),
                                ],
                                rhs=expT[h][0:psl, si * S + c0 : si * S + c0 + cl],
                                start=(si == 0),
                                stop=(si == 4),
                            )
                    # drain po fast: unnormalized copy + staged fast reciprocal
                    nc.vector.tensor_copy(
                        attnT_all[hoff : hoff + 64, (h // 2) * S : (h // 2 + 1) * S],
                        po[0:64, :],
                    )
                    rs = small_pool.tile([1, S], F32, tag="rs", name=f"rs{h % 2}")
                    nc.vector.tensor_copy(rs[:], po[64:65, :])
                    rinv[h] = small_pool.tile(
                        [1, S], F32, tag="rinv", name=f"rinv{h % 2}"
                    )
                    nc.vector.reciprocal_approx_fast(rinv[h][:], rs[:])
                    del expT[h]
                for h in heads:
                    hoff = (h % 2) * 64
                    rbc = small_pool.tile([128, S], F32, tag="rbc")
                    nc.gpsimd.partition_broadcast(rbc[:, :], rinv[h][0:1, :])
                    nc.vector.tensor_mul(
                        attnT_all[hoff : hoff + 64, (h // 2) * S : (h // 2 + 1) * S],
                        attnT_all[hoff : hoff + 64, (h // 2) * S : (h // 2 + 1) * S],
                        rbc[hoff : hoff + 64, :],
                    )

            # x DMAs for batches 0/1 queued ahead of the weight DMAs so
            # the PE transposes start immediately
            emit_x_dma(0, split=True)
            emit_x_dma(1, split=True)
            # b_qk is a slow element-gather (strided 4B reads); queue it after
            # the startup-critical x tiles -- it is first read ~25us in
            nc.sync.dma_start(
                b_qk[:], bqkv_ext[0 : 2 * D].rearrange("(m p) -> p m", p=128)
            )

            # weights -> SBUF bf16 via gpsimd casting DMA (full bandwidth,
            # no staging, runs on the SWDGE queue in parallel with the x DMAs
            # on the sync HWDGE queue); one tile per contraction block k so
            # the first qkT matmul only waits on block 0
            w_qkv_k = [
                cpool.tile([128, 3 * D], BF16, name=f"wqkv{k}") for k in range(NKT)
            ]
            w_fc_k = [cpool.tile([128, D], BF16, name=f"wfc{k}") for k in range(NKT)]
            for k in range(NKT):
                nc.gpsimd.dma_start(
                    w_qkv_k[k][:], wqkv_ext[k * 128 : (k + 1) * 128, :]
                )


            def emit_fc_si(b, attnT_all, si):
                s0, psl = S_TILES[si]
                pf = psW.tile([128, D], F32, tag="psW", name="pf")
                for k in range(NKT):
                    for c0, cl in CH_D:
                        nc.tensor.matmul(
                            pf[0:psl, c0 : c0 + cl],
                            lhsT=attnT_all[:, k * S + s0 : k * S + s0 + psl],
                            rhs=w_fc_k[k][:, c0 : c0 + cl],
                            start=(k == 0),
                            stop=(k == NKT - 1),
                        )
                osb = osb_pool.tile([128, D], F32, tag="osb")
                nc.vector.tensor_add(osb[0:psl, :], pf[0:psl, :], b_fc_bc[0:psl, :])
                nc.sync.dma_start(out_ext[b, s0 : s0 + psl, :], osb[0:psl, :])

            # ---- schedule ----
            # prologue: batch 0+1 transposes run during the weight DMAs;
            # steady state: all next-batch PE work (qkT m-tiles, v si-tiles,
            # xT transposes) is woven between scores si-groups of the current
            # batch so the PE never idles while ScalarE drains exp
            emit_xT(0)
            emit_xT(1)
            # only the qkT tiles scores pair 0/1 need up front; the rest are
            # woven into attention(0) ahead of their deadlines
            for m in (0, NKT, 1, NKT + 1):
                emit_qkT_mtile(0, m)
            # w_fc is first read by fc(0) ~100us in; queue its DMA behind
            # the startup-critical w_qkv stream
            for k in range(NKT):
                nc.gpsimd.dma_start(w_fc_k[k][:], wfc_ext[k * 128 : (k + 1) * 128, :])
            v_t = {}
            attnT_t = {}
            v_t[0] = emit_v(0)

            for b in range(NB):
                if b + 2 < NB:
                    emit_x_dma(b + 2)
                fillers = []
                # this batch's remaining qkT tiles, 2 per step, one step ahead
                # of the scores pair that reads them (pair p needs m=p, 6+p)
                for p in range(2, NKT):
                    fillers.append(
                        (lambda bb, mm: lambda: emit_qkT_mtile(bb, mm))(b, p)
                    )
                    fillers.append(
                        (lambda bb, mm: lambda: emit_qkT_mtile(bb, mm))(b, NKT + p)
                    )
                if b >= 1:
                    for si in range(5):
                        fillers.append(
                            (lambda bb, ss: lambda: emit_fc_si(bb, attnT_t[bb], ss))(
                                b - 1, si
                            )
                        )
                if b + 1 < NB:
                    for m in (0, NKT, 1, NKT + 1):
                        fillers.append(
                            (lambda bb, mm: lambda: emit_qkT_mtile(bb, mm))(b + 1, m)
                        )
                    fillers.append(
                        (lambda bb: lambda: v_t.__setitem__(bb, emit_v(bb)))(b + 1)
                    )
                if b + 2 < NB:
                    fillers.append((lambda bb: lambda: emit_xT(bb))(b + 2))
                fill_iter = iter(fillers)

                def try_fill():
                    f = next(fill_iter, None)
                    if f is not None:
                        f()

                v_all = v_t[b]
                attnT_t[b] = attnT_all = attnT_pool.tile(
                    [128, NKT * S], BF16, tag="attnT", name="attnT_all"
                )
                expT = {}
                for p in range(H // 2 + 1):
                    if p < H // 2:
                        emit_scores(p, qkT_t[b], expT, try_fill)
                    if p >= 1:
                        emit_attnv(p - 1, v_all, attnT_all, expT)
                # any leftover fillers run before the next batch
                for f in fill_iter:
                    f()

            for si in range(5):
                emit_fc_si(NB - 1, attnT_t[NB - 1], si)

    nc.compile()
    return nc


_NC_CACHE = None


def _get_nc():
    global _NC_CACHE
    if _NC_CACHE is None:
        _NC_CACHE = build_nc()
    return _NC_CACHE


def kernel(x, w_qkv, b_qkv, w_fc, b_fc, _collect=None):
    nc = _get_nc()
    x = np.ascontiguousarray(np.asarray(x, dtype=np.float32))
    w_qkv = np.ascontiguousarray(np.asarray(w_qkv, dtype=np.float32))
    b_qkv = np.ascontiguousarray(np.asarray(b_qkv, dtype=np.float32))
    w_fc = np.ascontiguousarray(np.asarray(w_fc, dtype=np.float32))
    b_fc = np.ascontiguousarray(np.asarray(b_fc, dtype=np.float32))
    in_maps = [
        {
            "x": x[i * NB : (i + 1) * NB],
            "w_qkv": w_qkv,
            "b_qkv": b_qkv,
            "w_fc": w_fc,
            "b_fc": b_fc,
        }
        for i in range(NCORES)
    ]
    kwargs = dict(_collect) if _collect else {}
    res = run_bass_kernel_spmd(nc, in_maps, core_ids=list(range(NCORES)), **kwargs)
    out = np.concatenate([res.results[i]["out"] for i in range(NCORES)], axis=0)
    if _collect is not None and isinstance(_collect, dict):
        _collect["result"] = res
    return out.astype(np.float32)


if __name__ == "__main__":
    xs = np.random.randn(B, S, D).astype(np.float32)
    lim = 1.0 / np.sqrt(D)
    rng = np.random.default_rng(0)
    wq = rng.uniform(-lim, lim, (D, 3 * D)).astype(np.float32)
    bq = rng.uniform(-lim, lim, (3 * D,)).astype(np.float32)
    wf = rng.uniform(-lim, lim, (D, D)).astype(np.float32)
    bf = rng.uniform(-lim, lim, (D,)).astype(np.float32)
    o = kernel(xs, wq, bq, wf, bf)
    print("out", o.shape, o.dtype)

